# revision 6
# baseline (speedup 1.0000x reference)
"""Causal linear attention (elu+1 feature map) on 8 Trainium2 NeuronCores.

Full inputs (n=2, l=2048, h=8, d=64) fp32 are sharded over the 16 (n,h)
head-sequences: core i handles pairs (2i, 2i+1). The elu(x)+1 feature maps
and all layout shuffles run on the HOST (numpy); the device does only the
memory/compute-heavy chunked causal scan (chunk C=128, state stride 2).

Per scan step s (chunks c0=2s, c1=2s+1), with Kf/Qf host-fmapped:

  at_ps = [AT(c0) p0|p1 | CROSS p0|p1 | AT(c1) p0|p1]     (2 matmuls: the
          c-major blocked qfb makes [AT(c0)|CROSS] one 512-col moving)
  atm   = tri-mask(at blocks 0,1,4,5 via one broadcast-mask DVE op)
          + CROSS copied by ScalarE
  out(c0) = atm(c0)^T @ Vaug_c0 + QfT_c0 @ S_sb
  out(c1) = atm(c1)^T @ Vaug_c1 + CROSS^T @ Vaug_c0 + QfT_c1 @ S_sb
  S_ps   += Kf_c^T @ Vaug_c  (both chunks, PSUM fp32, serial accumulator)

S_sb is an f16 ScalarE snapshot of S_ps with the cross-pair garbage blocks
kept at zero (zeros DMAed at startup, only diagonal blocks copied), so ONE
dense-qfT stationary serves both pairs' inter-chunk terms per chunk.

PSUM start=True is used on the first matmul touching each bank per group
(has_written semantics: later disjoint writers store, overlapping ones
accumulate) -- no zero-init matmuls. Out is written as f16 (num|den)/16;
the final num/den divide happens on the host.

Tile-framework dependency tracking is tile-granular, so EVERY DMA gets its
own SBUF tile (a reader then only waits for the transfers it actually
needs) and the mask / snapshot buffers are separate tiles (no false
DVE<->ScalarE serialization). The device issues no gpsimd/memset work at
all; every constant is uploaded.

Host layouts (f16, DMAs contiguous):
  qfb     : (128, 4096)  [(64p' + d), (256c + 128p + i)], zero unless p'=p
  qfT, kfT: (128, 2048)  [(64p + d), (128c + i)]
  kv      : (128, 4128)  [kf h0 | vaug h0 | kf h1 | vaug h1]
            kf cols (128c + 64p + d), vaug cols (130c + 65p + x), x=64 -> 1
  mz      : (128, 388)   [tri mask (j<=i) | zeros 260]
  o       : (128, 2080) f16 [i, (130c + 65p + x)]  (x<64 num/16, x=64 den/16)
"""
import numpy as np
from contextlib import ExitStack

import concourse.bacc as bacc
import concourse.bass as bass
import concourse.tile as tile
from concourse import mybir
from concourse.bass_utils import run_bass_kernel_spmd

N, L, H, D = 2, 2048, 8, 64
C = 128                 # chunk length
NCH = L // C            # 16 chunks
PAIRS = 2
QW = NCH * C            # 2048 cols (transposed layouts)
BW = NCH * PAIRS * C    # 4096 blocked qfb cols
KVH = 8 * C + 8 * (PAIRS * (D + 1))   # 2064: one half of the kv tensor
SW = PAIRS * (D + 1)    # 130: state cols [S_p0 | ksum_p0 | S_p1 | ksum_p1]
ATW = 6 * C             # at: [ATc0 p0|ATc0 p1|CROSS p0|CROSS p1|ATc1 p0|p1]
MZW = C + 2 * SW        # 388: mask + two zeroed snapshot buffers
OW = NCH * SW           # 2080 output cols
OSCALE = 1.0 / 16.0     # keeps num/den inside f16 range

f16 = mybir.dt.float16
f32 = mybir.dt.float32
OP = mybir.AluOpType


def build_kernel():
    nc = bacc.Bacc("TRN2", target_bir_lowering=False, debug=False, num_devices=8)
    qfb_d = nc.dram_tensor("qfb", (C, BW), f16, kind="ExternalInput").ap()
    qfT_d = nc.dram_tensor("qfT", (C, QW), f16, kind="ExternalInput").ap()
    kfT_d = nc.dram_tensor("kfT", (C, QW), f16, kind="ExternalInput").ap()
    kv_d = nc.dram_tensor("kv", (C, 2 * KVH), f16, kind="ExternalInput").ap()
    mz_d = nc.dram_tensor("mz", (C, MZW), f16, kind="ExternalInput").ap()
    o_d = nc.dram_tensor("o", (C, OW), f16, kind="ExternalOutput").ap()

    with tile.TileContext(nc) as tc, ExitStack() as ctx:
        consts = ctx.enter_context(tc.tile_pool(name="consts", bufs=1))
        sm_pool = ctx.enter_context(tc.tile_pool(name="sm", bufs=2))
        at_psum = ctx.enter_context(tc.tile_pool(name="at", bufs=2, space="PSUM"))
        out_psum = ctx.enter_context(tc.tile_pool(name="out", bufs=3, space="PSUM"))
        s_psum = ctx.enter_context(tc.tile_pool(name="sp", bufs=1, space="PSUM"))

        # one SBUF tile per DMA (dep tracking is tile-granular)
        qfb0 = consts.tile([C, BW // 4], f16, tag="qfb0")
        qfb1 = consts.tile([C, BW // 4], f16, tag="qfb1")
        qfb2 = consts.tile([C, BW // 4], f16, tag="qfb2")
        qfb3 = consts.tile([C, BW // 4], f16, tag="qfb3")
        qfb_t = [qfb0, qfb1, qfb2, qfb3]
        qfTa = consts.tile([C, QW // 2], f16, tag="qfTa")
        qfTb = consts.tile([C, QW // 2], f16, tag="qfTb")
        qfT_t = [qfTa, qfTb]
        kfTa = consts.tile([C, QW // 2], f16, tag="kfTa")
        kfTb = consts.tile([C, QW // 2], f16, tag="kfTb")
        kfT_t = [kfTa, kfTb]
        kva = consts.tile([C, KVH], f16, tag="kva")
        kvb = consts.tile([C, KVH], f16, tag="kvb")
        kv_t = [kva, kvb]
        maskt = consts.tile([C, C], f16)
        sb0 = consts.tile([C, SW], f16, tag="sb0")
        sb1 = consts.tile([C, SW], f16, tag="sb1")
        sbs = [sb0, sb1]
        ob = consts.tile([C, OW], f16)         # output staging

        def qfb_mv(c, w):            # at moving: chunks c..c+w-1 blocks
            return qfb_t[c // 4][:, (c % 4) * 2 * C:(c % 4 + w) * 2 * C]

        def qfT_st(c):               # snap stationary
            return qfT_t[c // 8][:, (c % 8) * C:(c % 8 + 1) * C]

        def kfT_st(c):               # at stationary
            return kfT_t[c // 8][:, (c % 8) * C:(c % 8 + 1) * C]

        def kf_st(c):                # S-update stationary
            return kv_t[c // 8][:, (c % 8) * C:(c % 8 + 1) * C]

        def vb_mv(c, p, w):          # vaug moving (w cols from pair p)
            base = 8 * C + (c % 8) * SW + p * (D + 1)
            return kv_t[c // 8][:, base:base + w]

        # input DMAs: sync + scalar both have HWDGE rings; ordered by need.
        nc.sync.dma_start(qfb_t[0], qfb_d[:, 0:BW // 4])
        nc.sync.dma_start(kfT_t[0], kfT_d[:, 0:QW // 2])
        nc.sync.dma_start(maskt, mz_d[:, 0:C])
        nc.sync.dma_start(qfb_t[1], qfb_d[:, BW // 4:BW // 2])
        nc.sync.dma_start(qfb_t[2], qfb_d[:, BW // 2:3 * BW // 4])
        nc.sync.dma_start(qfb_t[3], qfb_d[:, 3 * BW // 4:BW])
        nc.sync.dma_start(kfT_t[1], kfT_d[:, QW // 2:QW])
        nc.scalar.dma_start(kv_t[0], kv_d[:, 0:KVH])
        nc.scalar.dma_start(sb0, mz_d[:, C:C + SW])
        nc.scalar.dma_start(sb1, mz_d[:, C + SW:MZW])
        nc.scalar.dma_start(qfT_t[0], qfT_d[:, 0:QW // 2])
        nc.scalar.dma_start(kv_t[1], kv_d[:, KVH:2 * KVH])
        nc.scalar.dma_start(qfT_t[1], qfT_d[:, QW // 2:QW])

        # running state accumulator (off-diagonal blocks hold unread garbage)
        S_ps = s_psum.tile([C, SW], f32)

        def emit_at(s):
            """at matmuls + tri mask + cross copy for step s; returns atm."""
            c0, c1 = 2 * s, 2 * s + 1
            at_ps = at_psum.tile([C, ATW], f32, tag="at")
            atm = sm_pool.tile([C, ATW], f16, tag="atm")
            nc.tensor.matmul(at_ps[:, 0:4 * C], kfT_st(c0), qfb_mv(c0, 2),
                             start=True, stop=True)
            nc.tensor.matmul(at_ps[:, 4 * C:6 * C], kfT_st(c1), qfb_mv(c1, 1),
                             start=True, stop=True)
            # tri-mask blocks {0,1,4,5} in one op: broadcast 128x128 mask
            tri_in = bass.AP(tensor=at_ps.tensor, offset=at_ps.offset,
                             ap=[list(at_ps.ap[0]), [4 * C, 2], [C, 2], [1, C]])
            tri_out = bass.AP(tensor=atm.tensor, offset=atm.offset,
                              ap=[list(atm.ap[0]), [4 * C, 2], [C, 2], [1, C]])
            mask_b = bass.AP(tensor=maskt.tensor, offset=maskt.offset,
                             ap=[list(maskt.ap[0]), [0, 2], [0, 2], [1, C]])
            nc.vector.tensor_tensor(out=tri_out, in0=tri_in, in1=mask_b,
                                    op=OP.mult)
            nc.scalar.copy(atm[:, 2 * C:4 * C], at_ps[:, 2 * C:4 * C])
            return atm

        atm = emit_at(0)
        for s in range(8):
            c0, c1 = 2 * s, 2 * s + 1
            out_ps = out_psum.tile([C, 2 * SW], f32, tag="out")

            # inter-chunk terms from the snapshot (both pairs per matmul)
            if s > 0:
                sb = sbs[s % 2]
                nc.tensor.matmul(out_ps[:, 0:SW], qfT_st(c0), sb,
                                 start=True, stop=False, skip_group_check=True)
                nc.tensor.matmul(out_ps[:, SW:2 * SW], qfT_st(c1), sb,
                                 start=False, stop=False,
                                 skip_group_check=True)

            # state updates (skipped once no later chunk needs them)
            for c in (c0, c1):
                if c <= NCH - 3:
                    nc.tensor.matmul(
                        S_ps, kf_st(c), vb_mv(c, 0, SW),
                        start=(c == 0), stop=(c == NCH - 3),
                        skip_group_check=True)

            # f16 state snapshot for step s+1 (diagonal blocks only;
            # ScalarE on purpose -- DVE reads of the PE-accumulated S hang)
            if s < 7:
                nxt = sbs[(s + 1) % 2]
                nc.scalar.copy(nxt[0:64, 0:D + 1], S_ps[0:64, 0:D + 1])
                nc.scalar.copy(nxt[64:128, D + 1:SW], S_ps[64:128, D + 1:SW])

            # next step's at matmuls fill PE while DVE masks this step
            atm_next = emit_at(s + 1) if s < 7 else None

            # intra-chunk + cross contributions
            for p in range(PAIRS):
                vs = slice(p * (D + 1), (p + 1) * (D + 1))
                nc.tensor.matmul(        # intra c0
                    out_ps[:, vs], atm[:, p * C:(p + 1) * C],
                    vb_mv(c0, p, D + 1),
                    start=(s == 0 and p == 0), stop=False,
                    skip_group_check=True)
            for p in range(PAIRS):
                vs = slice(SW + p * (D + 1), SW + (p + 1) * (D + 1))
                nc.tensor.matmul(        # cross -> c1
                    out_ps[:, vs], atm[:, (2 + p) * C:(3 + p) * C],
                    vb_mv(c0, p, D + 1),
                    start=False, stop=False, skip_group_check=True)
            for p in range(PAIRS):
                vs = slice(SW + p * (D + 1), SW + (p + 1) * (D + 1))
                nc.tensor.matmul(        # intra c1
                    out_ps[:, vs], atm[:, (4 + p) * C:(5 + p) * C],
                    vb_mv(c1, p, D + 1),
                    start=False, stop=(p == PAIRS - 1),
                    skip_group_check=True)

            # scaled f16 staging copy; host does num/den
            nc.vector.tensor_scalar_mul(
                ob[:, s * 2 * SW:(s + 1) * 2 * SW], out_ps, OSCALE)
            if s % 2 == 1:
                k = s // 2
                nc.sync.dma_start(o_d[:, k * 4 * SW:(k + 1) * 4 * SW],
                                  ob[:, k * 4 * SW:(k + 1) * 4 * SW])
            atm = atm_next

    nc.compile()
    return nc


_nc_cache = None


def _get_nc():
    global _nc_cache
    if _nc_cache is None:
        _nc_cache = build_kernel()
    return _nc_cache


def _fmap_np(x):
    # elu(x) + 1 in fp32 on host
    return np.where(x < 0.0, np.exp(np.minimum(x, 0.0)), x + 1.0)


def _core_pairs(x, core):
    flat = np.asarray(x).transpose(0, 2, 1, 3).reshape(N * H, L, D)
    return flat[2 * core:2 * core + 2]          # (2, L, D) fp32


def _t_layout(xc):
    # (2, L, D) -> (128, 2048) [(64p + d), (128c + i)]
    return np.ascontiguousarray(
        xc.reshape(PAIRS, NCH, C, D).transpose(0, 3, 1, 2).reshape(C, QW)
    ).astype(np.float16)


def _mz_host():
    mz = np.zeros((C, MZW), np.float16)
    mz[:, 0:C] = np.triu(np.ones((C, C), np.float16))   # mask[j,i]=1 iff j<=i
    return mz


def make_in_maps(queries, keys, values):
    mz = _mz_host()
    in_maps = []
    for core in range(8):
        qf = _fmap_np(_core_pairs(queries, core).astype(np.float32))
        kf = _fmap_np(_core_pairs(keys, core).astype(np.float32))
        vc = _core_pairs(values, core).astype(np.float32)

        # blocked qfb, c-major: [(64p'+d), (256c+128p+i)], zero unless p'=p
        qft = qf.reshape(PAIRS, NCH, C, D).astype(np.float16)  # (p,c,i,d)
        qfb = np.zeros((C, NCH, PAIRS, C), np.float16)  # (row, c, p, i)
        for p in range(PAIRS):
            qfb[p * D:(p + 1) * D, :, p, :] = qft[p].transpose(2, 0, 1)
        qfb = qfb.reshape(C, BW)

        kf_nat = kf.reshape(PAIRS, NCH, C, D).transpose(2, 1, 0, 3) \
                   .reshape(C, NCH * PAIRS * D).astype(np.float16)
        va = np.ones((PAIRS, NCH, C, D + 1), np.float32)
        va[..., 0:D] = vc.reshape(PAIRS, NCH, C, D)
        vb_nat = va.transpose(2, 1, 0, 3).reshape(C, OW).astype(np.float16)
        kv = np.concatenate([
            kf_nat[:, 0:8 * C], vb_nat[:, 0:8 * SW],
            kf_nat[:, 8 * C:16 * C], vb_nat[:, 8 * SW:16 * SW],
        ], axis=1)
        in_maps.append({
            "qfb": np.ascontiguousarray(qfb),
            "qfT": _t_layout(qf),
            "kfT": _t_layout(kf),
            "kv": np.ascontiguousarray(kv),
            "mz": mz,
        })
    return in_maps


def _unpack_out(o_arr):
    # (128, 2080) f16 (num|den)/16 -> (2, L, D) fp32 normalized
    o4 = o_arr.astype(np.float32).reshape(C, NCH, PAIRS, D + 1)
    res = o4[..., 0:D] / o4[..., D:D + 1]
    return res.transpose(2, 1, 0, 3).reshape(PAIRS, L, D)


def kernel(queries, keys, values):
    nc = _get_nc()
    in_maps = make_in_maps(queries, keys, values)
    res = run_bass_kernel_spmd(nc, in_maps, core_ids=list(range(8)))
    out = np.zeros((N, L, H, D), np.float32)
    for core in range(8):
        oc = _unpack_out(res.results[core]["o"])
        for p in range(PAIRS):
            flat = 2 * core + p
            out[flat // H, :, flat % H, :] = oc[p]
    return out


# revision 8
# speedup vs baseline: 1.0876x; 1.0876x over previous
"""Causal linear attention (elu+1 feature map) on 8 Trainium2 NeuronCores.

Full inputs (n=2, l=2048, h=8, d=64) fp32 are sharded over the 16 (n,h)
head-sequences: core i handles pairs (2i, 2i+1). The elu(x)+1 feature maps
and all layout shuffles run on the HOST (numpy); the device does only the
memory/compute-heavy chunked causal scan (chunk C=128, state stride 2).

Per scan step s (chunks c0=2s, c1=2s+1), with Kf/Qf host-fmapped:

  at_ps = [AT(c0) p0|p1 | CROSS p0|p1 | AT(c1) p0|p1]     (2 matmuls: the
          c-major blocked qfb makes [AT(c0)|CROSS] one 512-col moving)
  atm   = tri-mask(at blocks 0,1,4,5 via one broadcast-mask DVE op)
          + CROSS copied by ScalarE
  out(c0) = atm(c0)^T @ Vaug_c0 + (16 QfT_c0) @ (S_sb / 16)
  out(c1) = atm(c1)^T @ Vaug_c1 + CROSS^T @ Vaug_c0 + (16 QfT_c1) @ S_sb/16
  S_ps   += Kf_c^T @ Vaug_c  (both chunks, PSUM fp32, serial accumulator)

The run is DMA-stream-bound (~90-150 GB/s per HWDGE ring, rings share the
SDMA engines), so the attention-weight path runs in fp8 e4m3: qfb, kfT and
qfT are uploaded as fp8, halving their bytes, and errors in the shared
qf*kf weights largely cancel between numerator and denominator. qfT is
pre-scaled by 16 and the S snapshot scaled by 1/16 (ksum ~2000 would
overflow fp8's 448 max; the scales cancel in the matmul). Input DMAs are
spread over four queues: sync + scalar (HWDGE) for qfb/kfT/mask and kv,
vector (SWDGE) for qfT + snapshot zeros, gpsimd (SWDGE) for outputs.

S_sb keeps its cross-pair garbage blocks at zero (zeros DMAed at startup,
only diagonal blocks copied by ScalarE -- DVE reads of the PE-accumulated
S bank hang), so ONE dense-qfT stationary serves both pairs per chunk.

PSUM start=True is used on the first matmul touching each bank per group
(has_written semantics: later disjoint writers store, overlapping ones
accumulate) -- no zero-init matmuls. Out is written as f16 (num|den)/16;
the final num/den divide happens on the host. Tile-framework dependency
tracking is tile-granular, so every DMA gets its own SBUF tile.

Host layouts (DMAs contiguous):
  qfb  f8 : (128, 4096)  [(64p' + d), (256c + 128p + i)], zero unless p'=p
  qfT  f8 : (128, 2048)  [(64p + d), (128c + i)], values * 16
  kfT  f8 : (128, 2048)  [(64p + d), (128c + i)]
  kv   f16: (128, 4128)  [kf h0 | vaug h0 | kf h1 | vaug h1]
            kf cols (128c + 64p + d), vaug cols (130c + 65p + x), x=64 -> 1
  mz   f16: (128, 128)   tri mask (j<=i)
  sbz  f8 : (128, 260)   zeros
  o    f16: (128, 2080)  [i, (130c + 65p + x)]  (x<64 num/16, x=64 den/16)
"""
import numpy as np
import ml_dtypes
from contextlib import ExitStack

import concourse.bacc as bacc
import concourse.bass as bass
import concourse.tile as tile
from concourse import mybir
from concourse.bass_utils import run_bass_kernel_spmd

N, L, H, D = 2, 2048, 8, 64
C = 128                 # chunk length
NCH = L // C            # 16 chunks
PAIRS = 2
QW = NCH * C            # 2048 cols (transposed layouts)
BW = NCH * PAIRS * C    # 4096 blocked qfb cols
KVH = 8 * C + 8 * (PAIRS * (D + 1))   # 2064: one half of the kv tensor
SW = PAIRS * (D + 1)    # 130: state cols [S_p0 | ksum_p0 | S_p1 | ksum_p1]
ATW = 6 * C             # at: [ATc0 p0|ATc0 p1|CROSS p0|CROSS p1|ATc1 p0|p1]
OW = NCH * SW           # 2080 output cols
OSCALE = 1.0 / 16.0     # keeps num/den inside f16 range
SSCALE = 1.0 / 16.0     # snapshot scale (ksum would overflow fp8 else)

f16 = mybir.dt.float16
f32 = mybir.dt.float32
f8 = mybir.dt.float8e4
np_f8 = ml_dtypes.float8_e4m3fn
OP = mybir.AluOpType


def build_kernel():
    nc = bacc.Bacc("TRN2", target_bir_lowering=False, debug=False, num_devices=8)
    qfb_d = nc.dram_tensor("qfb", (C, BW), f8, kind="ExternalInput").ap()
    qfT_d = nc.dram_tensor("qfT", (C, QW), f8, kind="ExternalInput").ap()
    kfT_d = nc.dram_tensor("kfT", (C, QW), f8, kind="ExternalInput").ap()
    kv_d = nc.dram_tensor("kv", (C, 2 * KVH), f16, kind="ExternalInput").ap()
    mz_d = nc.dram_tensor("mz", (C, C), f16, kind="ExternalInput").ap()
    sbz_d = nc.dram_tensor("sbz", (C, 2 * SW), f8, kind="ExternalInput").ap()
    o_d = nc.dram_tensor("o", (C, OW), f16, kind="ExternalOutput").ap()

    with tile.TileContext(nc) as tc, ExitStack() as ctx:
        consts = ctx.enter_context(tc.tile_pool(name="consts", bufs=1))
        sm_pool = ctx.enter_context(tc.tile_pool(name="sm", bufs=2))
        at_psum = ctx.enter_context(tc.tile_pool(name="at", bufs=2, space="PSUM"))
        out_psum = ctx.enter_context(tc.tile_pool(name="out", bufs=3, space="PSUM"))
        s_psum = ctx.enter_context(tc.tile_pool(name="sp", bufs=1, space="PSUM"))

        # one SBUF tile per DMA (dep tracking is tile-granular)
        qfb0 = consts.tile([C, BW // 4], f8, tag="qfb0")
        qfb1 = consts.tile([C, BW // 4], f8, tag="qfb1")
        qfb2 = consts.tile([C, BW // 4], f8, tag="qfb2")
        qfb3 = consts.tile([C, BW // 4], f8, tag="qfb3")
        qfb_t = [qfb0, qfb1, qfb2, qfb3]
        qfTs = consts.tile([C, QW], f8)
        kfTs = consts.tile([C, QW], f8)
        kva = consts.tile([C, KVH], f16, tag="kva")
        kvb = consts.tile([C, KVH], f16, tag="kvb")
        kv_t = [kva, kvb]
        maskt = consts.tile([C, C], f16)
        sb0 = consts.tile([C, SW], f8, tag="sb0")
        sb1 = consts.tile([C, SW], f8, tag="sb1")
        sbs = [sb0, sb1]
        ob = consts.tile([C, OW], f16)         # output staging

        def qfb_mv(c, w):            # at moving: chunks c..c+w-1 blocks
            return qfb_t[c // 4][:, (c % 4) * 2 * C:(c % 4 + w) * 2 * C]

        def kf_st(c):                # S-update stationary
            return kv_t[c // 8][:, (c % 8) * C:(c % 8 + 1) * C]

        def vb_mv(c, p, w):          # vaug moving (w cols from pair p)
            base = 8 * C + (c % 8) * SW + p * (D + 1)
            return kv_t[c // 8][:, base:base + w]

        # input DMAs spread over four queues, ordered by first use
        nc.sync.dma_start(qfb0, qfb_d[:, 0:BW // 4])
        nc.sync.dma_start(kfTs, kfT_d)
        nc.sync.dma_start(maskt, mz_d)
        nc.sync.dma_start(qfb1, qfb_d[:, BW // 4:BW // 2])
        nc.sync.dma_start(qfb2, qfb_d[:, BW // 2:3 * BW // 4])
        nc.sync.dma_start(qfb3, qfb_d[:, 3 * BW // 4:BW])
        nc.scalar.dma_start(kv_t[0], kv_d[:, 0:KVH])
        nc.scalar.dma_start(kv_t[1], kv_d[:, KVH:2 * KVH])
        nc.gpsimd.dma_start(sb0, sbz_d[:, 0:SW])
        nc.gpsimd.dma_start(sb1, sbz_d[:, SW:2 * SW])
        nc.gpsimd.dma_start(qfTs, qfT_d)

        # running state accumulator (off-diagonal blocks hold unread garbage)
        S_ps = s_psum.tile([C, SW], f32)

        def emit_at(s):
            """at matmuls + tri mask + cross copy for step s; returns atm."""
            c0, c1 = 2 * s, 2 * s + 1
            at_ps = at_psum.tile([C, ATW], f32, tag="at")
            atm = sm_pool.tile([C, ATW], f16, tag="atm")
            nc.tensor.matmul(at_ps[:, 0:4 * C],
                             kfTs[:, c0 * C:(c0 + 1) * C], qfb_mv(c0, 2),
                             start=True, stop=True)
            nc.tensor.matmul(at_ps[:, 4 * C:6 * C],
                             kfTs[:, c1 * C:(c1 + 1) * C], qfb_mv(c1, 1),
                             start=True, stop=True)
            # tri-mask blocks {0,1,4,5} in one op: broadcast 128x128 mask
            tri_in = bass.AP(tensor=at_ps.tensor, offset=at_ps.offset,
                             ap=[list(at_ps.ap[0]), [4 * C, 2], [C, 2], [1, C]])
            tri_out = bass.AP(tensor=atm.tensor, offset=atm.offset,
                              ap=[list(atm.ap[0]), [4 * C, 2], [C, 2], [1, C]])
            mask_b = bass.AP(tensor=maskt.tensor, offset=maskt.offset,
                             ap=[list(maskt.ap[0]), [0, 2], [0, 2], [1, C]])
            nc.vector.tensor_tensor(out=tri_out, in0=tri_in, in1=mask_b,
                                    op=OP.mult)
            nc.scalar.copy(atm[:, 2 * C:4 * C], at_ps[:, 2 * C:4 * C])
            return atm

        atm = emit_at(0)
        for s in range(8):
            c0, c1 = 2 * s, 2 * s + 1
            out_ps = out_psum.tile([C, 2 * SW], f32, tag="out")

            # inter-chunk terms from the snapshot (both pairs per matmul)
            if s > 0:
                sb = sbs[s % 2]
                nc.tensor.matmul(out_ps[:, 0:SW],
                                 qfTs[:, c0 * C:(c0 + 1) * C], sb,
                                 start=True, stop=False, skip_group_check=True)
                nc.tensor.matmul(out_ps[:, SW:2 * SW],
                                 qfTs[:, c1 * C:(c1 + 1) * C], sb,
                                 start=False, stop=False,
                                 skip_group_check=True)

            # state updates (skipped once no later chunk needs them)
            for c in (c0, c1):
                if c <= NCH - 3:
                    nc.tensor.matmul(
                        S_ps, kf_st(c), vb_mv(c, 0, SW),
                        start=(c == 0), stop=(c == NCH - 3),
                        skip_group_check=True)

            # scaled f8 state snapshot for step s+1 (diagonal blocks only;
            # ScalarE on purpose -- DVE reads of the PE-accumulated S hang)
            if s < 7:
                nxt = sbs[(s + 1) % 2]
                nc.scalar.mul(nxt[0:64, 0:D + 1], S_ps[0:64, 0:D + 1], SSCALE)
                nc.scalar.mul(nxt[64:128, D + 1:SW], S_ps[64:128, D + 1:SW],
                              SSCALE)

            # next step's at matmuls fill PE while DVE masks this step
            atm_next = emit_at(s + 1) if s < 7 else None

            # intra-chunk + cross contributions
            for p in range(PAIRS):
                vs = slice(p * (D + 1), (p + 1) * (D + 1))
                nc.tensor.matmul(        # intra c0
                    out_ps[:, vs], atm[:, p * C:(p + 1) * C],
                    vb_mv(c0, p, D + 1),
                    start=(s == 0 and p == 0), stop=False,
                    skip_group_check=True)
            for p in range(PAIRS):
                vs = slice(SW + p * (D + 1), SW + (p + 1) * (D + 1))
                nc.tensor.matmul(        # cross -> c1
                    out_ps[:, vs], atm[:, (2 + p) * C:(3 + p) * C],
                    vb_mv(c0, p, D + 1),
                    start=False, stop=False, skip_group_check=True)
            for p in range(PAIRS):
                vs = slice(SW + p * (D + 1), SW + (p + 1) * (D + 1))
                nc.tensor.matmul(        # intra c1
                    out_ps[:, vs], atm[:, (4 + p) * C:(5 + p) * C],
                    vb_mv(c1, p, D + 1),
                    start=False, stop=(p == PAIRS - 1),
                    skip_group_check=True)

            # scaled f16 staging copy; host does num/den
            nc.vector.tensor_scalar_mul(
                ob[:, s * 2 * SW:(s + 1) * 2 * SW], out_ps, OSCALE)
            if s % 2 == 1:
                k = s // 2
                nc.gpsimd.dma_start(o_d[:, k * 4 * SW:(k + 1) * 4 * SW],
                                    ob[:, k * 4 * SW:(k + 1) * 4 * SW])
            atm = atm_next

    nc.compile()
    return nc


_nc_cache = None


def _get_nc():
    global _nc_cache
    if _nc_cache is None:
        _nc_cache = build_kernel()
    return _nc_cache


def _fmap_np(x):
    # elu(x) + 1 in fp32 on host
    return np.where(x < 0.0, np.exp(np.minimum(x, 0.0)), x + 1.0)


def _core_pairs(x, core):
    flat = np.asarray(x).transpose(0, 2, 1, 3).reshape(N * H, L, D)
    return flat[2 * core:2 * core + 2]          # (2, L, D) fp32


def _t_layout(xc, dtype, scale=1.0):
    # (2, L, D) -> (128, 2048) [(64p + d), (128c + i)]
    return np.ascontiguousarray(
        (xc * scale).reshape(PAIRS, NCH, C, D).transpose(0, 3, 1, 2)
        .reshape(C, QW)
    ).astype(dtype)


def make_in_maps(queries, keys, values):
    mz = np.triu(np.ones((C, C), np.float16))   # mask[j,i]=1 iff j<=i
    sbz = np.zeros((C, 2 * SW), np_f8)
    in_maps = []
    for core in range(8):
        qf = _fmap_np(_core_pairs(queries, core).astype(np.float32))
        kf = _fmap_np(_core_pairs(keys, core).astype(np.float32))
        vc = _core_pairs(values, core).astype(np.float32)

        # blocked qfb, c-major: [(64p'+d), (256c+128p+i)], zero unless p'=p
        qft = qf.reshape(PAIRS, NCH, C, D).astype(np_f8)  # (p,c,i,d)
        qfb = np.zeros((C, NCH, PAIRS, C), np_f8)  # (row, c, p, i)
        for p in range(PAIRS):
            qfb[p * D:(p + 1) * D, :, p, :] = qft[p].transpose(2, 0, 1)
        qfb = qfb.reshape(C, BW)

        kf_nat = kf.reshape(PAIRS, NCH, C, D).transpose(2, 1, 0, 3) \
                   .reshape(C, NCH * PAIRS * D).astype(np.float16)
        va = np.ones((PAIRS, NCH, C, D + 1), np.float32)
        va[..., 0:D] = vc.reshape(PAIRS, NCH, C, D)
        vb_nat = va.transpose(2, 1, 0, 3).reshape(C, OW).astype(np.float16)
        kv = np.concatenate([
            kf_nat[:, 0:8 * C], vb_nat[:, 0:8 * SW],
            kf_nat[:, 8 * C:16 * C], vb_nat[:, 8 * SW:16 * SW],
        ], axis=1)
        in_maps.append({
            "qfb": np.ascontiguousarray(qfb),
            "qfT": _t_layout(qf, np_f8, 1.0 / SSCALE),
            "kfT": _t_layout(kf, np_f8),
            "kv": np.ascontiguousarray(kv),
            "mz": mz,
            "sbz": sbz,
        })
    return in_maps


def _unpack_out(o_arr):
    # (128, 2080) f16 (num|den)/16 -> (2, L, D) fp32 normalized
    o4 = o_arr.astype(np.float32).reshape(C, NCH, PAIRS, D + 1)
    res = o4[..., 0:D] / o4[..., D:D + 1]
    return res.transpose(2, 1, 0, 3).reshape(PAIRS, L, D)


def kernel(queries, keys, values):
    nc = _get_nc()
    in_maps = make_in_maps(queries, keys, values)
    res = run_bass_kernel_spmd(nc, in_maps, core_ids=list(range(8)))
    out = np.zeros((N, L, H, D), np.float32)
    for core in range(8):
        oc = _unpack_out(res.results[core]["o"])
        for p in range(PAIRS):
            flat = 2 * core + p
            out[flat // H, :, flat % H, :] = oc[p]
    return out


# revision 13
# speedup vs baseline: 1.1239x; 1.0334x over previous
"""Causal linear attention (elu+1 feature map) on 8 Trainium2 NeuronCores.

Full inputs (n=2, l=2048, h=8, d=64) fp32 are sharded over the 16 (n,h)
head-sequences: core i handles pairs (2i, 2i+1). The elu(x)+1 feature maps
and all layout shuffles run on the HOST (numpy); the device does only the
memory/compute-heavy chunked causal scan (chunk C=128, state stride 2).

Per scan step s (chunks c0=2s, c1=2s+1), with Kf/Qf host-fmapped:

  at_ps = [AT(c0) p0|p1 | CROSS p0|p1 | AT(c1) p0|p1]     (2 matmuls: the
          c-major blocked qfb makes [AT(c0)|CROSS] one 512-col moving)
  atm   = tri-mask(at blocks 0,1,4,5 via one broadcast-mask DVE op)
          + CROSS copied by ScalarE
  out(c0) = atm(c0)^T @ Vaug_c0 + (16 QfT_c0) @ (S_sb / 16)
  out(c1) = atm(c1)^T @ Vaug_c1 + CROSS^T @ Vaug_c0 + (16 QfT_c1) @ S_sb/16
  S_ps   += Kf_c^T @ Vaug_c  (both chunks, PSUM fp32, serial accumulator)

The run is DMA-stream-bound (~90-150 GB/s per HWDGE ring, rings share the
SDMA engines), so the attention-weight path runs in fp8 e4m3: qfb, kfT and
qfT are uploaded as fp8, halving their bytes, and errors in the shared
qf*kf weights largely cancel between numerator and denominator. qfT is
pre-scaled by 16 and the S snapshot scaled by 1/16 (ksum ~2000 would
overflow fp8's 448 max; the scales cancel in the matmul). Input DMAs are
spread over four queues: sync + scalar (HWDGE) for qfb/kfT/mask and kv,
vector (SWDGE) for qfT + snapshot zeros, gpsimd (SWDGE) for outputs.

S_sb keeps its cross-pair garbage blocks at zero (zeros DMAed at startup,
only diagonal blocks copied by ScalarE -- DVE reads of the PE-accumulated
S bank hang), so ONE dense-qfT stationary serves both pairs per chunk.

PSUM start=True is used on the first matmul touching each bank per group
(has_written semantics: later disjoint writers store, overlapping ones
accumulate) -- no zero-init matmuls. Out is written as f16 (num|den)/16;
the final num/den divide happens on the host. Tile-framework dependency
tracking is tile-granular, so every DMA gets its own SBUF tile.

Host layouts (DMAs contiguous):
  qfb  f8 : (128, 4096)  [(64p' + d), (256c + 128p + i)], zero unless p'=p
  qfT  f8 : (128, 2048)  [(64p + d), (128c + i)], values * 16
  kfT  f8 : (128, 2048)  [(64p + d), (128c + i)]
  kv   f16: (128, 4128)  [kf h0 | vaug h0 | kf h1 | vaug h1]
            kf cols (128c + 64p + d), vaug cols (130c + 65p + x), x=64 -> 1
  mz   f16: (128, 128)   tri mask (j<=i)
  sbz  f8 : (128, 260)   zeros
  o    f16: (128, 2080)  [i, (130c + 65p + x)]  (x<64 num/16, x=64 den/16)
"""
import numpy as np
import ml_dtypes
from contextlib import ExitStack

import concourse.bacc as bacc
import concourse.bass as bass
import concourse.tile as tile
from concourse import mybir
from concourse.bass_utils import run_bass_kernel_spmd

N, L, H, D = 2, 2048, 8, 64
C = 128                 # chunk length
NCH = L // C            # 16 chunks
PAIRS = 2
QW = NCH * C            # 2048 cols (transposed layouts)
BW = NCH * PAIRS * C    # 4096 blocked qfb cols
KVH = 8 * C + 8 * (PAIRS * (D + 1))   # 2064: one half of the kv tensor
SW = PAIRS * (D + 1)    # 130: state cols [S_p0 | ksum_p0 | S_p1 | ksum_p1]
ATW = 6 * C             # at: [ATc0 p0|ATc0 p1|CROSS p0|CROSS p1|ATc1 p0|p1]
OW = NCH * SW           # 2080 output cols
OSCALE = 1.0 / 16.0     # keeps num/den inside f16 range
SSCALE = 1.0 / 16.0     # snapshot scale (ksum would overflow fp8 else)

f16 = mybir.dt.float16
f32 = mybir.dt.float32
f8 = mybir.dt.float8e4
np_f8 = ml_dtypes.float8_e4m3fn
OP = mybir.AluOpType


def build_kernel():
    nc = bacc.Bacc("TRN2", target_bir_lowering=False, debug=False, num_devices=8)
    qfb_d = nc.dram_tensor("qfb", (C, BW), f8, kind="ExternalInput").ap()
    qfT_d = nc.dram_tensor("qfT", (C, QW), f8, kind="ExternalInput").ap()
    kfT_d = nc.dram_tensor("kfT", (C, QW), f8, kind="ExternalInput").ap()
    kv_d = nc.dram_tensor("kv", (C, 2 * KVH), f16, kind="ExternalInput").ap()
    mz_d = nc.dram_tensor("mz", (C, C), f16, kind="ExternalInput").ap()
    sbz_d = nc.dram_tensor("sbz", (C, 2 * SW), f8, kind="ExternalInput").ap()
    o_d = nc.dram_tensor("o", (C, OW), f16, kind="ExternalOutput").ap()

    with tile.TileContext(nc) as tc, ExitStack() as ctx:
        consts = ctx.enter_context(tc.tile_pool(name="consts", bufs=1))
        sm_pool = ctx.enter_context(tc.tile_pool(name="sm", bufs=2))
        at_psum = ctx.enter_context(tc.tile_pool(name="at", bufs=2, space="PSUM"))
        out_psum = ctx.enter_context(tc.tile_pool(name="out", bufs=3, space="PSUM"))
        s_psum = ctx.enter_context(tc.tile_pool(name="sp", bufs=1, space="PSUM"))

        # one SBUF tile per DMA (dep tracking is tile-granular)
        qfb0 = consts.tile([C, BW // 4], f8, tag="qfb0")
        qfb1 = consts.tile([C, BW // 4], f8, tag="qfb1")
        qfb2 = consts.tile([C, BW // 4], f8, tag="qfb2")
        qfb3 = consts.tile([C, BW // 4], f8, tag="qfb3")
        qfb_t = [qfb0, qfb1, qfb2, qfb3]
        qfTs = consts.tile([C, QW], f8)
        kfTa = consts.tile([C, QW // 2], f8, tag="kfTa")
        kfTb = consts.tile([C, QW // 2], f8, tag="kfTb")
        kfT_t = [kfTa, kfTb]
        kva = consts.tile([C, KVH], f16, tag="kva")
        kvb = consts.tile([C, KVH], f16, tag="kvb")
        kv_t = [kva, kvb]
        maskt = consts.tile([C, C], f16)
        sb0 = consts.tile([C, SW], f8, tag="sb0")
        sb1 = consts.tile([C, SW], f8, tag="sb1")
        sbs = [sb0, sb1]
        ob = consts.tile([C, OW], f16)         # output staging

        def qfb_mv(c, w):            # at moving: chunks c..c+w-1 blocks
            return qfb_t[c // 4][:, (c % 4) * 2 * C:(c % 4 + w) * 2 * C]

        def kfT_st(c):               # at stationary
            return kfT_t[c // 8][:, (c % 8) * C:(c % 8 + 1) * C]

        def kf_st(c):                # S-update stationary
            return kv_t[c // 8][:, (c % 8) * C:(c % 8 + 1) * C]

        def vb_mv(c, p, w):          # vaug moving (w cols from pair p)
            base = 8 * C + (c % 8) * SW + p * (D + 1)
            return kv_t[c // 8][:, base:base + w]

        # input DMAs spread over three queues, ordered by first use
        nc.sync.dma_start(kfTa, kfT_d[:, 0:QW // 2])
        nc.sync.dma_start(qfb0, qfb_d[:, 0:BW // 4])
        nc.sync.dma_start(maskt, mz_d)
        nc.sync.dma_start(kfTb, kfT_d[:, QW // 2:QW])
        nc.sync.dma_start(qfb1, qfb_d[:, BW // 4:BW // 2])
        nc.sync.dma_start(qfb2, qfb_d[:, BW // 2:3 * BW // 4])
        nc.sync.dma_start(qfb3, qfb_d[:, 3 * BW // 4:BW])
        nc.scalar.dma_start(kv_t[0], kv_d[:, 0:KVH])
        nc.scalar.dma_start(qfTs, qfT_d)
        nc.scalar.dma_start(kv_t[1], kv_d[:, KVH:2 * KVH])
        nc.gpsimd.dma_start(sb0, sbz_d[:, 0:SW])
        nc.gpsimd.dma_start(sb1, sbz_d[:, SW:2 * SW])

        # running state accumulator (off-diagonal blocks hold unread garbage)
        S_ps = s_psum.tile([C, SW], f32)

        def emit_at(s):
            """at matmuls + tri mask + cross copy for step s; returns atm."""
            c0, c1 = 2 * s, 2 * s + 1
            at_ps = at_psum.tile([C, ATW], f32, tag="at")
            atm = sm_pool.tile([C, ATW], f16, tag="atm")
            nc.tensor.matmul(at_ps[:, 0:4 * C], kfT_st(c0), qfb_mv(c0, 2),
                             start=True, stop=True)
            nc.tensor.matmul(at_ps[:, 4 * C:6 * C], kfT_st(c1), qfb_mv(c1, 1),
                             start=True, stop=True)
            # tri-mask blocks {0,1,4,5} in one op: broadcast 128x128 mask
            tri_in = bass.AP(tensor=at_ps.tensor, offset=at_ps.offset,
                             ap=[list(at_ps.ap[0]), [4 * C, 2], [C, 2], [1, C]])
            tri_out = bass.AP(tensor=atm.tensor, offset=atm.offset,
                              ap=[list(atm.ap[0]), [4 * C, 2], [C, 2], [1, C]])
            mask_b = bass.AP(tensor=maskt.tensor, offset=maskt.offset,
                             ap=[list(maskt.ap[0]), [0, 2], [0, 2], [1, C]])
            nc.vector.tensor_tensor(out=tri_out, in0=tri_in, in1=mask_b,
                                    op=OP.mult)
            nc.scalar.copy(atm[:, 2 * C:4 * C], at_ps[:, 2 * C:4 * C])
            return atm

        atm = emit_at(0)
        for s in range(8):
            c0, c1 = 2 * s, 2 * s + 1
            out_ps = out_psum.tile([C, 2 * SW], f32, tag="out")

            # inter-chunk terms from the snapshot (both pairs per matmul)
            if s > 0:
                sb = sbs[s % 2]
                nc.tensor.matmul(out_ps[:, 0:SW],
                                 qfTs[:, c0 * C:(c0 + 1) * C], sb,
                                 start=True, stop=False, skip_group_check=True)
                nc.tensor.matmul(out_ps[:, SW:2 * SW],
                                 qfTs[:, c1 * C:(c1 + 1) * C], sb,
                                 start=False, stop=False,
                                 skip_group_check=True)

            # state updates (skipped once no later chunk needs them)
            for c in (c0, c1):
                if c <= NCH - 3:
                    nc.tensor.matmul(
                        S_ps, kf_st(c), vb_mv(c, 0, SW),
                        start=(c == 0), stop=(c == NCH - 3),
                        skip_group_check=True)

            # scaled f8 state snapshot for step s+1 (diagonal blocks only;
            # ScalarE on purpose -- DVE reads of the PE-accumulated S hang)
            if s < 7:
                nxt = sbs[(s + 1) % 2]
                nc.scalar.mul(nxt[0:64, 0:D + 1], S_ps[0:64, 0:D + 1], SSCALE)
                nc.scalar.mul(nxt[64:128, D + 1:SW], S_ps[64:128, D + 1:SW],
                              SSCALE)

            # next step's at matmuls fill PE while DVE masks this step
            atm_next = emit_at(s + 1) if s < 7 else None

            # intra-chunk + cross contributions
            for p in range(PAIRS):
                vs = slice(p * (D + 1), (p + 1) * (D + 1))
                nc.tensor.matmul(        # intra c0
                    out_ps[:, vs], atm[:, p * C:(p + 1) * C],
                    vb_mv(c0, p, D + 1),
                    start=(s == 0 and p == 0), stop=False,
                    skip_group_check=True)
            for p in range(PAIRS):
                vs = slice(SW + p * (D + 1), SW + (p + 1) * (D + 1))
                nc.tensor.matmul(        # cross -> c1
                    out_ps[:, vs], atm[:, (2 + p) * C:(3 + p) * C],
                    vb_mv(c0, p, D + 1),
                    start=False, stop=False, skip_group_check=True)
            for p in range(PAIRS):
                vs = slice(SW + p * (D + 1), SW + (p + 1) * (D + 1))
                nc.tensor.matmul(        # intra c1
                    out_ps[:, vs], atm[:, (4 + p) * C:(5 + p) * C],
                    vb_mv(c1, p, D + 1),
                    start=False, stop=(p == PAIRS - 1),
                    skip_group_check=True)

            # scaled f16 staging copy; host does num/den
            nc.vector.tensor_scalar_mul(
                ob[:, s * 2 * SW:(s + 1) * 2 * SW], out_ps, OSCALE)
            nc.sync.dma_start(o_d[:, s * 2 * SW:(s + 1) * 2 * SW],
                              ob[:, s * 2 * SW:(s + 1) * 2 * SW])
            atm = atm_next

    nc.compile()
    return nc


_nc_cache = None


def _get_nc():
    global _nc_cache
    if _nc_cache is None:
        _nc_cache = build_kernel()
    return _nc_cache


def _fmap_np(x):
    # elu(x) + 1 in fp32 on host
    return np.where(x < 0.0, np.exp(np.minimum(x, 0.0)), x + 1.0)


def _core_pairs(x, core):
    flat = np.asarray(x).transpose(0, 2, 1, 3).reshape(N * H, L, D)
    return flat[2 * core:2 * core + 2]          # (2, L, D) fp32


def _t_layout(xc, dtype, scale=1.0):
    # (2, L, D) -> (128, 2048) [(64p + d), (128c + i)]
    return np.ascontiguousarray(
        (xc * scale).reshape(PAIRS, NCH, C, D).transpose(0, 3, 1, 2)
        .reshape(C, QW)
    ).astype(dtype)


def make_in_maps(queries, keys, values):
    mz = np.triu(np.ones((C, C), np.float16))   # mask[j,i]=1 iff j<=i
    sbz = np.zeros((C, 2 * SW), np_f8)
    in_maps = []
    for core in range(8):
        qf = _fmap_np(_core_pairs(queries, core).astype(np.float32))
        kf = _fmap_np(_core_pairs(keys, core).astype(np.float32))
        vc = _core_pairs(values, core).astype(np.float32)

        # blocked qfb, c-major: [(64p'+d), (256c+128p+i)], zero unless p'=p
        qft = qf.reshape(PAIRS, NCH, C, D).astype(np_f8)  # (p,c,i,d)
        qfb = np.zeros((C, NCH, PAIRS, C), np_f8)  # (row, c, p, i)
        for p in range(PAIRS):
            qfb[p * D:(p + 1) * D, :, p, :] = qft[p].transpose(2, 0, 1)
        qfb = qfb.reshape(C, BW)

        kf_nat = kf.reshape(PAIRS, NCH, C, D).transpose(2, 1, 0, 3) \
                   .reshape(C, NCH * PAIRS * D).astype(np.float16)
        va = np.ones((PAIRS, NCH, C, D + 1), np.float32)
        va[..., 0:D] = vc.reshape(PAIRS, NCH, C, D)
        vb_nat = va.transpose(2, 1, 0, 3).reshape(C, OW).astype(np.float16)
        kv = np.concatenate([
            kf_nat[:, 0:8 * C], vb_nat[:, 0:8 * SW],
            kf_nat[:, 8 * C:16 * C], vb_nat[:, 8 * SW:16 * SW],
        ], axis=1)
        in_maps.append({
            "qfb": np.ascontiguousarray(qfb),
            "qfT": _t_layout(qf, np_f8, 1.0 / SSCALE),
            "kfT": _t_layout(kf, np_f8),
            "kv": np.ascontiguousarray(kv),
            "mz": mz,
            "sbz": sbz,
        })
    return in_maps


def _unpack_out(o_arr):
    # (128, 2080) f16 (num|den)/16 -> (2, L, D) fp32 normalized
    o4 = o_arr.astype(np.float32).reshape(C, NCH, PAIRS, D + 1)
    res = o4[..., 0:D] / o4[..., D:D + 1]
    return res.transpose(2, 1, 0, 3).reshape(PAIRS, L, D)


def kernel(queries, keys, values):
    nc = _get_nc()
    in_maps = make_in_maps(queries, keys, values)
    res = run_bass_kernel_spmd(nc, in_maps, core_ids=list(range(8)))
    out = np.zeros((N, L, H, D), np.float32)
    for core in range(8):
        oc = _unpack_out(res.results[core]["o"])
        for p in range(PAIRS):
            flat = 2 * core + p
            out[flat // H, :, flat % H, :] = oc[p]
    return out


# revision 17
# speedup vs baseline: 1.1862x; 1.0554x over previous
"""Causal linear attention (elu+1 feature map) on 8 Trainium2 NeuronCores.

Full inputs (n=2, l=2048, h=8, d=64) fp32 are sharded over the 16 (n,h)
head-sequences: core i handles pairs (2i, 2i+1). The elu(x)+1 feature maps
and all layout shuffles run on the HOST (numpy); the device does only the
memory/compute-heavy chunked causal scan (chunk C=128, state stride 2).

Per scan step s (chunks c0=2s, c1=2s+1), with Kf/Qf host-fmapped:

  at_ps = [AT(c0) p0|p1 | CROSS p0|p1 | AT(c1) p0|p1]     (2 matmuls: the
          c-major blocked qfb makes [AT(c0)|CROSS] one 512-col moving)
  atm   = tri-mask(at blocks 0,1,4,5 via one broadcast-mask DVE op)
          + CROSS copied by ScalarE
  out(c) = atm(c)^T @ Vaug_c (+ CROSS^T @ Vaug_c0 for c1)
           + per-pair qfb_c^T @ S_sb                      (4 snap matmuls)
  S_ps  += Kf_c^T @ Vaug_c  (both chunks, PSUM fp32, serial accumulator)

S_sb is a single full ScalarE fp8 copy of S_ps per step (ScalarE on
purpose -- DVE reads of the PE-accumulated S bank hang). Its cross-pair
garbage blocks are killed by the zero off-pair rows of the blocked qfb
stationary, so no zeroing or separate dense qfT upload is needed. The
vaug "ones" column is 1/16 so the running ksum stays inside fp8 range
(max 448); the host divide folds the extra 16.

The run is DMA-stream-bound (the two HWDGE rings share 16 SDMA engines at
~240 GB/s aggregate, and a DMA's completion is the slowest engine's last
packet, so extra bytes and extra queues directly delay compute). The
attention-weight path (qfb, kfT) is fp8 e4m3 -- errors in the shared
qf*kf weights largely cancel between numerator and denominator. Every
DMA gets its own SBUF tile (dep tracking is tile-granular).

PSUM start=True is used on the first matmul touching each bank per group
(has_written semantics: later disjoint writers store, overlapping ones
accumulate) -- no zero-init matmuls. Out is written as f16
(num|den_true/16)/16; the host computes num/(16*den).

Host layouts (DMAs contiguous):
  qfb  f8 : (128, 4096)  [(64p' + d), (256c + 128p + i)], zero unless p'=p
  kfT  f8 : (128, 2048)  [(64p + d), (128c + i)]
  kv   f16: (128, 4128)  [kf h0 | vaug h0 | kf h1 | vaug h1]
            kf cols (128c + 64p + d), vaug cols (130c + 65p + x), x=64->1/16
  mz   f16: (128, 128)   tri mask (j<=i)
  o    f16: (128, 2080)  [i, (130c + 65p + x)]
"""
import numpy as np
import ml_dtypes
from contextlib import ExitStack

import concourse.bacc as bacc
import concourse.bass as bass
import concourse.tile as tile
from concourse import mybir
from concourse.bass_utils import run_bass_kernel_spmd

N, L, H, D = 2, 2048, 8, 64
C = 128                 # chunk length
NCH = L // C            # 16 chunks
PAIRS = 2
QW = NCH * C            # 2048 cols (transposed layouts)
BW = NCH * PAIRS * C    # 4096 blocked qfb cols
KVH = 8 * C + 8 * (PAIRS * (D + 1))   # 2064: one half of the kv tensor
SW = PAIRS * (D + 1)    # 130: state cols [S_p0 | ksum_p0 | S_p1 | ksum_p1]
ATW = 6 * C             # at: [ATc0 p0|ATc0 p1|CROSS p0|CROSS p1|ATc1 p0|p1]
OW = NCH * SW           # 2080 output cols
OSCALE = 1.0 / 16.0     # keeps num/den inside f16 range
DSCALE = 1.0 / 16.0     # vaug ones column (keeps ksum inside fp8 range)
VSCALE = 1.0 / 4.0      # vaug value columns (keeps the S snapshot < fp8 448)

f16 = mybir.dt.float16
f32 = mybir.dt.float32
f8 = mybir.dt.float8e4
np_f8 = ml_dtypes.float8_e4m3fn
OP = mybir.AluOpType


def build_kernel():
    nc = bacc.Bacc("TRN2", target_bir_lowering=False, debug=False, num_devices=8)
    qfb_d = nc.dram_tensor("qfb", (C, BW), f8, kind="ExternalInput").ap()
    kfT_d = nc.dram_tensor("kfT", (C, QW), f8, kind="ExternalInput").ap()
    kv_d = nc.dram_tensor("kv", (C, 2 * KVH), f16, kind="ExternalInput").ap()
    mz_d = nc.dram_tensor("mz", (C, C), f16, kind="ExternalInput").ap()
    o_d = nc.dram_tensor("o", (C, OW), f16, kind="ExternalOutput").ap()

    with tile.TileContext(nc) as tc, ExitStack() as ctx:
        consts = ctx.enter_context(tc.tile_pool(name="consts", bufs=1))
        sm_pool = ctx.enter_context(tc.tile_pool(name="sm", bufs=2))
        at_psum = ctx.enter_context(tc.tile_pool(name="at", bufs=2, space="PSUM"))
        out_psum = ctx.enter_context(tc.tile_pool(name="out", bufs=3, space="PSUM"))
        s_psum = ctx.enter_context(tc.tile_pool(name="sp", bufs=1, space="PSUM"))

        # one SBUF tile per DMA (dep tracking is tile-granular)
        qfb0 = consts.tile([C, BW // 4], f8, tag="qfb0")
        qfb1 = consts.tile([C, BW // 4], f8, tag="qfb1")
        qfb2 = consts.tile([C, BW // 4], f8, tag="qfb2")
        qfb3 = consts.tile([C, BW // 4], f8, tag="qfb3")
        qfb_t = [qfb0, qfb1, qfb2, qfb3]
        kfTa = consts.tile([C, QW // 2], f8, tag="kfTa")
        kfTb = consts.tile([C, QW // 2], f8, tag="kfTb")
        kfT_t = [kfTa, kfTb]
        kva = consts.tile([C, KVH], f16, tag="kva")
        kvb = consts.tile([C, KVH], f16, tag="kvb")
        kv_t = [kva, kvb]
        maskt = consts.tile([C, C], f16)
        sb0 = consts.tile([C, SW], f8, tag="sb0")   # fully overwritten
        sb1 = consts.tile([C, SW], f8, tag="sb1")   # each step
        sbs = [sb0, sb1]
        ob = consts.tile([C, OW], f16)              # output staging

        def qfb_mv(c, w):            # at moving: chunks c..c+w-1 blocks
            return qfb_t[c // 4][:, (c % 4) * 2 * C:(c % 4 + w) * 2 * C]

        def qfb_st(c, p):            # snap stationary (zero off-pair rows)
            base = (c % 4) * 2 * C + p * C
            return qfb_t[c // 4][:, base:base + C]

        def kfT_st(c):               # at stationary
            return kfT_t[c // 8][:, (c % 8) * C:(c % 8 + 1) * C]

        def kf_st(c):                # S-update stationary
            return kv_t[c // 8][:, (c % 8) * C:(c % 8 + 1) * C]

        def vb_mv(c, p, w):          # vaug moving (w cols from pair p)
            base = 8 * C + (c % 8) * SW + p * (D + 1)
            return kv_t[c // 8][:, base:base + w]

        # input DMAs on the two HWDGE rings, ordered by first use
        nc.sync.dma_start(kfTa, kfT_d[:, 0:QW // 2])
        nc.sync.dma_start(qfb0, qfb_d[:, 0:BW // 4])
        nc.sync.dma_start(maskt, mz_d)
        nc.sync.dma_start(qfb1, qfb_d[:, BW // 4:BW // 2])
        nc.sync.dma_start(kfTb, kfT_d[:, QW // 2:QW])
        nc.sync.dma_start(qfb2, qfb_d[:, BW // 2:3 * BW // 4])
        nc.sync.dma_start(qfb3, qfb_d[:, 3 * BW // 4:BW])
        nc.scalar.dma_start(kv_t[0], kv_d[:, 0:KVH])
        nc.scalar.dma_start(kv_t[1], kv_d[:, KVH:2 * KVH])

        # running state accumulator (off-diagonal blocks hold unread garbage)
        S_ps = s_psum.tile([C, SW], f32)

        def emit_at(s):
            """at matmuls + tri mask + cross copy for step s; returns atm."""
            c0, c1 = 2 * s, 2 * s + 1
            at_ps = at_psum.tile([C, ATW], f32, tag="at")
            atm = sm_pool.tile([C, ATW], f16, tag="atm")
            nc.tensor.matmul(at_ps[:, 0:4 * C], kfT_st(c0), qfb_mv(c0, 2),
                             start=True, stop=True)
            nc.tensor.matmul(at_ps[:, 4 * C:6 * C], kfT_st(c1), qfb_mv(c1, 1),
                             start=True, stop=True)
            # tri-mask blocks {0,1,4,5} in one op: broadcast 128x128 mask
            tri_in = bass.AP(tensor=at_ps.tensor, offset=at_ps.offset,
                             ap=[list(at_ps.ap[0]), [4 * C, 2], [C, 2], [1, C]])
            tri_out = bass.AP(tensor=atm.tensor, offset=atm.offset,
                              ap=[list(atm.ap[0]), [4 * C, 2], [C, 2], [1, C]])
            mask_b = bass.AP(tensor=maskt.tensor, offset=maskt.offset,
                             ap=[list(maskt.ap[0]), [0, 2], [0, 2], [1, C]])
            nc.vector.tensor_tensor(out=tri_out, in0=tri_in, in1=mask_b,
                                    op=OP.mult)
            nc.scalar.copy(atm[:, 2 * C:4 * C], at_ps[:, 2 * C:4 * C])
            return atm

        atm = emit_at(0)
        for s in range(8):
            c0, c1 = 2 * s, 2 * s + 1
            out_ps = out_psum.tile([C, 2 * SW], f32, tag="out")

            # inter-chunk terms from the snapshot, per (chunk, pair):
            # the qfb stationary's zero rows kill the snapshot garbage
            if s > 0:
                sb = sbs[s % 2]
                for dj, c in ((0, c0), (1, c1)):
                    for p in range(PAIRS):
                        lo = dj * SW + p * (D + 1)
                        nc.tensor.matmul(
                            out_ps[:, lo:lo + D + 1], qfb_st(c, p),
                            sb[:, p * (D + 1):(p + 1) * (D + 1)],
                            start=(dj == 0 and p == 0), stop=False,
                            skip_group_check=True)

            # state updates (skipped once no later chunk needs them)
            for c in (c0, c1):
                if c <= NCH - 3:
                    nc.tensor.matmul(
                        S_ps, kf_st(c), vb_mv(c, 0, SW),
                        start=(c == 0), stop=(c == NCH - 3),
                        skip_group_check=True)

            # fp8 state snapshot for step s+1: ONE full copy, garbage and all
            if s < 7:
                nc.scalar.copy(sbs[(s + 1) % 2], S_ps)

            # next step's at matmuls fill PE while DVE masks this step
            atm_next = emit_at(s + 1) if s < 7 else None

            # intra-chunk + cross contributions
            for p in range(PAIRS):
                vs = slice(p * (D + 1), (p + 1) * (D + 1))
                nc.tensor.matmul(        # intra c0
                    out_ps[:, vs], atm[:, p * C:(p + 1) * C],
                    vb_mv(c0, p, D + 1),
                    start=(s == 0 and p == 0), stop=False,
                    skip_group_check=True)
            for p in range(PAIRS):
                vs = slice(SW + p * (D + 1), SW + (p + 1) * (D + 1))
                nc.tensor.matmul(        # cross -> c1
                    out_ps[:, vs], atm[:, (2 + p) * C:(3 + p) * C],
                    vb_mv(c0, p, D + 1),
                    start=False, stop=False, skip_group_check=True)
            for p in range(PAIRS):
                vs = slice(SW + p * (D + 1), SW + (p + 1) * (D + 1))
                nc.tensor.matmul(        # intra c1
                    out_ps[:, vs], atm[:, (4 + p) * C:(5 + p) * C],
                    vb_mv(c1, p, D + 1),
                    start=False, stop=(p == PAIRS - 1),
                    skip_group_check=True)

            # scaled f16 staging copy; host does the divide
            nc.vector.tensor_scalar_mul(
                ob[:, s * 2 * SW:(s + 1) * 2 * SW], out_ps, OSCALE)
            nc.sync.dma_start(o_d[:, s * 2 * SW:(s + 1) * 2 * SW],
                              ob[:, s * 2 * SW:(s + 1) * 2 * SW])
            atm = atm_next

    nc.compile()
    return nc


_nc_cache = None


def _get_nc():
    global _nc_cache
    if _nc_cache is None:
        _nc_cache = build_kernel()
    return _nc_cache


def _fmap_np(x):
    # elu(x) + 1 in fp32 on host
    return np.where(x < 0.0, np.exp(np.minimum(x, 0.0)), x + 1.0)


def _core_pairs(x, core):
    flat = np.asarray(x).transpose(0, 2, 1, 3).reshape(N * H, L, D)
    return flat[2 * core:2 * core + 2]          # (2, L, D) fp32


def _t_layout(xc, dtype):
    # (2, L, D) -> (128, 2048) [(64p + d), (128c + i)]
    return np.ascontiguousarray(
        xc.reshape(PAIRS, NCH, C, D).transpose(0, 3, 1, 2).reshape(C, QW)
    ).astype(dtype)


def make_in_maps(queries, keys, values):
    mz = np.triu(np.ones((C, C), np.float16))   # mask[j,i]=1 iff j<=i
    in_maps = []
    for core in range(8):
        qf = _fmap_np(_core_pairs(queries, core).astype(np.float32))
        kf = _fmap_np(_core_pairs(keys, core).astype(np.float32))
        vc = _core_pairs(values, core).astype(np.float32)

        # blocked qfb, c-major: [(64p'+d), (256c+128p+i)], zero unless p'=p
        qft = qf.reshape(PAIRS, NCH, C, D).astype(np_f8)  # (p,c,i,d)
        qfb = np.zeros((C, NCH, PAIRS, C), np_f8)  # (row, c, p, i)
        for p in range(PAIRS):
            qfb[p * D:(p + 1) * D, :, p, :] = qft[p].transpose(2, 0, 1)
        qfb = qfb.reshape(C, BW)

        kf_nat = kf.reshape(PAIRS, NCH, C, D).transpose(2, 1, 0, 3) \
                   .reshape(C, NCH * PAIRS * D).astype(np.float16)
        va = np.full((PAIRS, NCH, C, D + 1), DSCALE, np.float32)
        va[..., 0:D] = vc.reshape(PAIRS, NCH, C, D) * VSCALE
        vb_nat = va.transpose(2, 1, 0, 3).reshape(C, OW).astype(np.float16)
        kv = np.concatenate([
            kf_nat[:, 0:8 * C], vb_nat[:, 0:8 * SW],
            kf_nat[:, 8 * C:16 * C], vb_nat[:, 8 * SW:16 * SW],
        ], axis=1)
        in_maps.append({
            "qfb": np.ascontiguousarray(qfb),
            "kfT": _t_layout(kf, np_f8),
            "kv": np.ascontiguousarray(kv),
            "mz": mz,
        })
    return in_maps


def _unpack_out(o_arr):
    # (128, 2080) f16 -> (2, L, D) fp32, undoing the DSCALE/VSCALE split
    o4 = o_arr.astype(np.float32).reshape(C, NCH, PAIRS, D + 1)
    res = o4[..., 0:D] * (DSCALE / VSCALE / o4[..., D:D + 1])
    return res.transpose(2, 1, 0, 3).reshape(PAIRS, L, D)


def kernel(queries, keys, values):
    nc = _get_nc()
    in_maps = make_in_maps(queries, keys, values)
    res = run_bass_kernel_spmd(nc, in_maps, core_ids=list(range(8)))
    out = np.zeros((N, L, H, D), np.float32)
    for core in range(8):
        oc = _unpack_out(res.results[core]["o"])
        for p in range(PAIRS):
            flat = 2 * core + p
            out[flat // H, :, flat % H, :] = oc[p]
    return out
